# revision 25
# baseline (speedup 1.0000x reference)
"""Trainium2 Bass kernel for nn_Autograd4bitQuantLinear (4-bit quant linear).

Computes out = x @ dequant4(qweight, scales, zeros) + bias where
  x:       (4, 2048, 4096) f32
  qweight: (512, 11008)    i32  (8 nibbles packed per int32 along rows)
  scales:  (11008, 1)      f32
  zeros:   (11008, 1)      f32
  bias:    (11008,)        f32
  out:     (4, 2048, 11008) f32

Strategy (tensor-parallel over 8 NeuronCores, column-sharded out_features):
  - Each core owns 1376 output columns; x is replicated.
  - On-device dequant: nibble-unpack qweight int32 (DVE shift/and with
    per-partition shift amounts), fold scale/zero (W = q * s - z), W bf16.
  - W is NOT kept SBUF-resident: it is produced just-in-time for chunk 0
    (DVE unpack at ~1.6us/k-tile vs PE consumption 1.73us/k-tile), spilled
    to a DRAM scratch, and re-streamed per chunk afterwards. This frees
    ~65KB/partition of SBUF for the x-tile pool.
  - x is cast f32->bf16 by SWDGE cast-DMAs into DRAM scratch (chunk 0 in
    8 column slices so the first transpose lands ~8us in), then
    DMA-transposed (xbar) into SBUF as [128, 1024] k-major tiles. The
    68-tile xt pool double-buffers whole chunks, so each chunk's 32
    transposes run entirely during the previous chunk's compute (the
    transpose issue+wait cost of ~2.6us each made transpose supply the
    bottleneck of earlier versions).
  - PE: chunks of 1024 rows, group-major k-outer with mt-inner over all
    8 PSUM banks: out[m, n] accumulated over 32 k-tiles (bf16 -> f32).
  - Queue roles: sync = transposes; scalar = output stores; gpsimd
    (SWDGE) = casts, qweight loads, W spills/loads, broadcasts. (Issuing
    transposes on the scalar queue corrupts data on HW - keep them on
    sync only.)
  - Epilogue per (group, mt): psum + bias (DVE) -> SBUF -> store.
"""

import sys

sys.path.insert(0, "/opt/trn_rl_repo")

import numpy as np

import concourse.bass as bass
import concourse.mybir as mybir
from concourse import bacc
from concourse.tile import TileContext
from concourse.tile_rust import add_dep_helper


dt = mybir.dt
AL = mybir.AluOpType

P = 128
IN = 4096  # contraction dim (in_features)
OUT = 11008  # out_features
M_ROWS = 8192  # 4 * 2048
NCORES = 8
NSH = OUT // NCORES  # 1376 output columns per core
KT = IN // P  # 32 k-tiles
MC = 1024  # rows per chunk
# n-chunks within the per-core shard; each must fit one PSUM bank (<=512 f32)
N_CHUNKS = ((0, 512), (512, 512), (1024, 352))
XT_BUFS = 68  # [128, 1024] bf16 tiles; two full chunks + slack
W_BUFS = 10  # streaming W tiles in flight per group


def build(m_rows=M_ROWS, debug=False):
    """Build + compile the single-core Tile program (SPMD: same on all cores)."""
    assert m_rows % MC == 0
    nc = bacc.Bacc(None, target_bir_lowering=False, debug=debug)

    x_d = nc.dram_tensor("x", [m_rows, IN], dt.float32, kind="ExternalInput")
    qw_d = nc.dram_tensor("qw", [IN, NSH], dt.int32, kind="ExternalInput")
    s_d = nc.dram_tensor("scales", [NSH], dt.float32, kind="ExternalInput")
    z_d = nc.dram_tensor("zeros", [NSH], dt.float32, kind="ExternalInput")
    b_d = nc.dram_tensor("bias", [NSH], dt.float32, kind="ExternalInput")
    shamt_d = nc.dram_tensor("shamt", [P, 1], dt.int32, kind="ExternalInput")
    out_d = nc.dram_tensor("out", [m_rows, NSH], dt.float32, kind="ExternalOutput")

    n_chunks = m_rows // MC
    mt_per_chunk = MC // P

    with TileContext(nc) as tc:
        with (
            tc.tile_pool(name="singles", bufs=1) as singles,
            tc.tile_pool(name="w", bufs=W_BUFS) as wpool,
            tc.tile_pool(name="unpack", bufs=4) as upool,
            tc.tile_pool(name="xbf0", bufs=2, space="DRAM") as xbf0pool,
            tc.tile_pool(name="wdr", bufs=1, space="DRAM") as wdrpool,
            tc.tile_pool(name="xt", bufs=XT_BUFS) as xtpool,
            tc.tile_pool(name="osb", bufs=2) as opool,
            tc.tile_pool(name="ps", bufs=1, space="PSUM") as pspool,
        ):
            # ---- constants ----
            s_rep = singles.tile([P, NSH], dt.float32, tag="s_rep")
            nc.gpsimd.dma_start(out=s_rep[:], in_=s_d[None, :].to_broadcast([P, NSH]))
            z_rep = singles.tile([P, NSH], dt.float32, tag="z_rep")
            nc.gpsimd.dma_start(out=z_rep[:], in_=z_d[None, :].to_broadcast([P, NSH]))
            shamt = singles.tile([P, 1], dt.int32, tag="shamt")
            nc.scalar.dma_start(out=shamt[:], in_=shamt_d[:])
            mask = singles.tile([P, 1], dt.int32, tag="mask")
            nc.vector.memset(mask[:], 15)

            b_rep = singles.tile([P, NSH], dt.float32, tag="b_rep")

            wdram = wdrpool.tile([IN, NSH], dt.bfloat16, tag="wdram", name="wdram")

            last_xpose = {}
            xbf_slices = {}

            def cast_chunk(c):
                """8 column-slice cast DMAs per chunk: bounds the SWDGE ring
                head-of-line delay for latency-sensitive W/qw loads at ~5us."""
                r0 = c * MC
                slices = []
                for j in range(8):
                    t = xbf0pool.tile([MC, 512], dt.bfloat16, tag=f"xbf_{j}",
                                      name=f"xbf{c}_{j}")
                    ci = nc.gpsimd.dma_start(
                        out=t[:], in_=x_d[r0 : r0 + MC, j * 512 : (j + 1) * 512]
                    )
                    if c - 2 in last_xpose:
                        add_dep_helper(
                            ci.ins,
                            last_xpose[c - 2].ins,
                            sync=True,
                            reason="throttle x cast chain",
                        )
                    slices.append(t)
                xbf_slices[c] = slices

            def transpose_chunk(c):
                """32 xbar transposes -> [128, 1024] tiles on the sync queue."""
                xts = []
                for k in range(KT):
                    xt = xtpool.tile([P, MC], dt.bfloat16, tag="xt", name="xt")
                    j = k // 4
                    src = xbf_slices[c][j][
                        :, k * 128 - j * 512 : (k + 1) * 128 - j * 512
                    ]
                    ti = nc.sync.dma_start(out=xt[:], in_=src, transpose=True)
                    xts.append(xt)
                last_xpose[c] = ti
                return xts

            cast_chunk(0)
            nc.gpsimd.dma_start(out=b_rep[:], in_=b_d[None, :].to_broadcast([P, NSH]))

            # ---- W dequant (chunk-0 JIT) + spill to DRAM ----
            def unpack_group(i):
                """Returns wtiles for chunk 0's group-i pass; spills to wdram."""
                o, wd = N_CHUNKS[i]
                qts = []
                for k in range(KT):
                    qt = upool.tile([P, wd], dt.int32, tag="qt", name="qt")
                    nc.gpsimd.dma_start(
                        out=qt[:], in_=qw_d[k * P : (k + 1) * P, o : o + wd]
                    )
                    qts.append(qt)
                wtiles = {}
                for k in range(KT):
                    nib = upool.tile([P, wd], dt.int32, tag="nib", name="nib",
                                     bufs=1)
                    nc.vector.scalar_tensor_tensor(
                        nib[:],
                        qts[k][:],
                        shamt[:, 0:1],
                        mask[:, 0:1].to_broadcast([P, wd]),
                        AL.logical_shift_right,
                        AL.bitwise_and,
                    )
                    ws = upool.tile([P, wd], dt.float32, tag="ws", name="ws",
                                    bufs=1)
                    nc.vector.tensor_tensor(
                        ws[:], nib[:], s_rep[:, o : o + wd], AL.mult
                    )
                    wt = wpool.tile([P, wd], dt.bfloat16, tag=f"w{i}",
                                    name=f"w{i}_{k}")
                    nc.vector.tensor_tensor(
                        wt[:], ws[:], z_rep[:, o : o + wd], AL.subtract
                    )
                    nc.gpsimd.dma_start(
                        out=wdram[k * P : (k + 1) * P, o : o + wd], in_=wt[:]
                    )
                    wtiles[k] = wt
                return wtiles

            W_HEAD = W_BUFS  # head prefetches a full rotation's worth

            def wload_head(i):
                """Prefetch the first W_HEAD W tiles of group i on the scalar
                queue (issued a group or chunk ahead; their pool buffers are
                free by then, so they never block the queue)."""
                o, wd = N_CHUNKS[i]
                wtiles = {}
                for k in range(W_HEAD):
                    wt = wpool.tile([P, wd], dt.bfloat16, tag=f"w{i}",
                                    name=f"w{i}_{k}")
                    nc.scalar.dma_start(
                        out=wt[:], in_=wdram[k * P : (k + 1) * P, o : o + wd]
                    )
                    wtiles[k] = wt
                return wtiles

            def wload_tail(i, wtiles):
                """JIT-stream the remaining W tiles of group i on gpsimd;
                paced by pool rotation against the PE's k-consumption."""
                o, wd = N_CHUNKS[i]
                for k in range(W_HEAD, KT):
                    wt = wpool.tile([P, wd], dt.bfloat16, tag=f"w{i}",
                                    name=f"w{i}_{k}")
                    nc.gpsimd.dma_start(
                        out=wt[:], in_=wdram[k * P : (k + 1) * P, o : o + wd]
                    )
                    wtiles[k] = wt
                return wtiles

            def chunk_group(c, i, xts, wtiles):
                o, wd = N_CHUNKS[i]
                pss = [
                    pspool.tile([P, wd], dt.float32, tag=f"ps{m}", name=f"ps{m}")
                    for m in range(mt_per_chunk)
                ]
                for k in range(KT):
                    for mt in range(mt_per_chunk):
                        nc.tensor.matmul(
                            pss[mt][:],
                            xts[k][:, mt * P : (mt + 1) * P],
                            wtiles[k][:],
                            start=(k == 0),
                            stop=(k == KT - 1),
                        )
                for mt in range(mt_per_chunk):
                    ob = opool.tile([P, wd], dt.float32, tag=f"ob{i}",
                                    name=f"ob{i}")
                    nc.vector.tensor_tensor(
                        ob[:], pss[mt][:], b_rep[:, o : o + wd], AL.add
                    )
                    row = c * MC + mt * P
                    nc.scalar.dma_start(
                        out=out_d[row : row + P, o : o + wd], in_=ob[:]
                    )

            # ---- program ----
            xts_cur = transpose_chunk(0)
            if n_chunks > 1:
                cast_chunk(1)
            xts_next = None
            head0 = None
            for i in range(3):
                wt0 = unpack_group(i)
                chunk_group(0, i, xts_cur, wt0)
                if i == 1:
                    if n_chunks > 1:
                        xts_next = transpose_chunk(1)
                    if n_chunks > 2:
                        cast_chunk(2)
                    if n_chunks > 1:
                        head0 = wload_head(0)

            for c in range(1, n_chunks):
                xts = xts_next
                wts0 = wload_tail(0, head0)
                head1 = wload_head(1)
                chunk_group(c, 0, xts, wts0)
                wts1 = wload_tail(1, head1)
                head2 = wload_head(2)
                chunk_group(c, 1, xts, wts1)
                if c + 1 < n_chunks:
                    xts_next = transpose_chunk(c + 1)
                if c + 2 < n_chunks:
                    cast_chunk(c + 2)
                if c + 1 < n_chunks:
                    head0 = wload_head(0)
                wts2 = wload_tail(2, head2)
                chunk_group(c, 2, xts, wts2)

    nc.compile()
    return nc


_SHAMT = (4 * (np.arange(P, dtype=np.int32) % 8)).reshape(P, 1)


def make_in_maps(x2d, qweight, scales, zeros, bias):
    """Per-core input maps (host-side sharding / layout prep only)."""
    in_maps = []
    for c in range(NCORES):
        sl = slice(c * NSH, (c + 1) * NSH)
        in_maps.append(
            {
                "x": x2d,
                "qw": np.ascontiguousarray(
                    np.repeat(qweight[:, sl], 8, axis=0)
                ),
                "scales": np.ascontiguousarray(scales[sl, 0]),
                "zeros": np.ascontiguousarray(zeros[sl, 0]),
                "bias": np.ascontiguousarray(bias[sl]),
                "shamt": _SHAMT,
            }
        )
    return in_maps


_NC_CACHE = {}


def _get_nc(m_rows):
    if m_rows not in _NC_CACHE:
        _NC_CACHE[m_rows] = build(m_rows)
    return _NC_CACHE[m_rows]


def run_spmd(x2d, qweight, scales, zeros, bias, trace=False, **kwargs):
    """Run on the 8 NeuronCores; returns (out2d [8192, 11008] f32, results)."""
    from concourse.bass_utils import run_bass_kernel_spmd

    m_rows = x2d.shape[0]
    nc = _get_nc(m_rows)
    in_maps = make_in_maps(x2d, qweight, scales, zeros, bias)
    res = run_bass_kernel_spmd(
        nc, in_maps, list(range(NCORES)), trace=trace, **kwargs
    )
    outs = [res.results[c]["out"] for c in range(NCORES)]
    out2d = np.concatenate(outs, axis=1)
    return out2d, res


def kernel(x, qweight, scales, zeros, bias):
    x = np.asarray(x, dtype=np.float32)
    qweight = np.asarray(qweight, dtype=np.int32)
    scales = np.asarray(scales, dtype=np.float32)
    zeros = np.asarray(zeros, dtype=np.float32)
    bias = np.asarray(bias, dtype=np.float32)

    b, s, k_in = x.shape
    x2d = np.ascontiguousarray(x.reshape(b * s, k_in))
    out2d, _ = run_spmd(x2d, qweight, scales, zeros, bias)
    return out2d.reshape(b, s, OUT)


# revision 27
# speedup vs baseline: 1.0689x; 1.0689x over previous
"""Trainium2 Bass kernel for nn_Autograd4bitQuantLinear (4-bit quant linear).

Computes out = x @ dequant4(qweight, scales, zeros) + bias where
  x:       (4, 2048, 4096) f32
  qweight: (512, 11008)    i32  (8 nibbles packed per int32 along rows)
  scales:  (11008, 1)      f32
  zeros:   (11008, 1)      f32
  bias:    (11008,)        f32
  out:     (4, 2048, 11008) f32

Strategy (tensor-parallel over 8 NeuronCores, column-sharded out_features):
  - Each core owns 1376 output columns; x is replicated.
  - On-device dequant: nibble-unpack qweight int32 (DVE shift/and with
    per-partition shift amounts), fold scale/zero (W = q * s - z); W bf16
    [4096, 1376] stays SBUF-resident (86KB/partition), produced just in
    time for the first compute phase (DVE ~1.6us/k-tile vs PE consumption
    1.73us/k-tile in the 8-bank phase).
  - x staging is uniform 512-row units: 8 column-slice cast DMAs
    (f32->bf16, SWDGE) into DRAM scratch, then 32 xbar transposes
    [512,128] -> SBUF [128,512] per unit, all on the sync queue. Casts
    run two units ahead and are emitted before any paced loads
    (anti-convoy); transpose dependencies are pre-satisfied so the DMA
    rings can run many transposes in parallel (JIT-gated transposes
    serialize to ~6us each - the failure mode of earlier versions).
  - PE phase 0 (rows 0:1024): group-major k-outer, mt-inner over all 8
    PSUM banks, matching the W-unpack rate. Steady chunks (512 rows):
    k-outer over 4 PSUM banks, bank parity alternating per chunk. The
    512-row chunk size is what lets the xt pool (80 x [128,512] tiles)
    double-buffer whole chunks next to the resident W.
  - Queue roles: sync = transposes only; scalar = output stores; gpsimd
    (SWDGE) = casts, qweight loads, broadcasts. (Transposes on the
    scalar queue corrupt data on HW; keep them on sync.)
  - Epilogue per (group, mt): psum + bias (DVE) -> SBUF -> store.
"""

import sys

sys.path.insert(0, "/opt/trn_rl_repo")

import numpy as np

import concourse.bass as bass
import concourse.mybir as mybir
from concourse import bacc
from concourse.tile import TileContext
from concourse.tile_rust import add_dep_helper


dt = mybir.dt
AL = mybir.AluOpType

P = 128
IN = 4096  # contraction dim (in_features)
OUT = 11008  # out_features
M_ROWS = 8192  # 4 * 2048
NCORES = 8
NSH = OUT // NCORES  # 1376 output columns per core
KT = IN // P  # 32 k-tiles
MU = 512  # x staging unit rows (casts + transposes)
# n-chunks within the per-core shard; each must fit one PSUM bank (<=512 f32)
N_CHUNKS = ((0, 512), (512, 512), (1024, 352))
XT_BUFS = 80  # [128, 512] bf16 tiles; phase0 holds 64, steady chunks 32


def build(m_rows=M_ROWS, debug=False):
    """Build + compile the single-core Tile program (SPMD: same on all cores)."""
    assert m_rows % MU == 0 and m_rows >= 2 * MU
    nc = bacc.Bacc(None, target_bir_lowering=False, debug=debug)

    x_d = nc.dram_tensor("x", [m_rows, IN], dt.float32, kind="ExternalInput")
    qw_d = nc.dram_tensor("qw", [IN, NSH], dt.int32, kind="ExternalInput")
    s_d = nc.dram_tensor("scales", [NSH], dt.float32, kind="ExternalInput")
    z_d = nc.dram_tensor("zeros", [NSH], dt.float32, kind="ExternalInput")
    b_d = nc.dram_tensor("bias", [NSH], dt.float32, kind="ExternalInput")
    shamt_d = nc.dram_tensor("shamt", [P, 1], dt.int32, kind="ExternalInput")
    out_d = nc.dram_tensor("out", [m_rows, NSH], dt.float32, kind="ExternalOutput")

    n_units = m_rows // MU  # 16

    with TileContext(nc) as tc:
        with (
            tc.tile_pool(name="singles", bufs=1) as singles,
            tc.tile_pool(name="w", bufs=KT) as wpool,
            tc.tile_pool(name="unpack", bufs=4) as upool,
            tc.tile_pool(name="xbf", bufs=2, space="DRAM") as xbfpool,
            tc.tile_pool(name="xt", bufs=XT_BUFS) as xtpool,
            tc.tile_pool(name="osb", bufs=2) as opool,
            tc.tile_pool(name="ps", bufs=1, space="PSUM") as pspool,
        ):
            # ---- constants ----
            s_rep = singles.tile([P, NSH], dt.float32, tag="s_rep")
            nc.gpsimd.dma_start(out=s_rep[:], in_=s_d[None, :].to_broadcast([P, NSH]))
            z_rep = singles.tile([P, NSH], dt.float32, tag="z_rep")
            nc.gpsimd.dma_start(out=z_rep[:], in_=z_d[None, :].to_broadcast([P, NSH]))
            shamt = singles.tile([P, 1], dt.int32, tag="shamt")
            nc.scalar.dma_start(out=shamt[:], in_=shamt_d[:])
            mask = singles.tile([P, 1], dt.int32, tag="mask")
            nc.vector.memset(mask[:], 15)
            b_rep = singles.tile([P, NSH], dt.float32, tag="b_rep")

            last_xpose = {}
            xbf_slices = {}

            def cast_unit(u):
                """8 column-slice cast DMAs per 512-row unit (bounds ring
                head-of-line delay; slices run in parallel across rings)."""
                r0 = u * MU
                slices = []
                for j in range(8):
                    t = xbfpool.tile([MU, 512], dt.bfloat16, tag=f"xbf_{j}",
                                     name=f"xbf{u}_{j}")
                    ci = nc.gpsimd.dma_start(
                        out=t[:], in_=x_d[r0 : r0 + MU, j * 512 : (j + 1) * 512]
                    )
                    if u - 2 in last_xpose:
                        add_dep_helper(
                            ci.ins,
                            last_xpose[u - 2].ins,
                            sync=True,
                            reason="throttle x cast chain",
                        )
                    slices.append(t)
                xbf_slices[u] = slices

            def transpose_unit(u):
                """32 xbar transposes -> [128, 512] tiles on the sync queue."""
                xts = []
                for k in range(KT):
                    xt = xtpool.tile([P, MU], dt.bfloat16, tag="xt", name="xt")
                    j = k // 4
                    src = xbf_slices[u][j][
                        :, k * 128 - j * 512 : (k + 1) * 128 - j * 512
                    ]
                    ti = nc.sync.dma_start(out=xt[:], in_=src, transpose=True)
                    xts.append(xt)
                last_xpose[u] = ti
                return xts

            # ---- W dequant (JIT for phase 0) ----
            wtiles = {}  # (i, k) -> [P, w_i] bf16 tile

            def unpack_group(i):
                o, wd = N_CHUNKS[i]
                qts = []
                for k in range(KT):
                    qt = upool.tile([P, wd], dt.int32, tag="qt", name="qt")
                    nc.gpsimd.dma_start(
                        out=qt[:], in_=qw_d[k * P : (k + 1) * P, o : o + wd]
                    )
                    qts.append(qt)
                for k in range(KT):
                    nib = upool.tile([P, wd], dt.int32, tag="nib", name="nib",
                                     bufs=1)
                    nc.vector.scalar_tensor_tensor(
                        nib[:],
                        qts[k][:],
                        shamt[:, 0:1],
                        mask[:, 0:1].to_broadcast([P, wd]),
                        AL.logical_shift_right,
                        AL.bitwise_and,
                    )
                    ws = upool.tile([P, wd], dt.float32, tag="ws", name="ws",
                                    bufs=1)
                    nc.vector.tensor_tensor(
                        ws[:], nib[:], s_rep[:, o : o + wd], AL.mult
                    )
                    wt = wpool.tile([P, wd], dt.bfloat16, tag=f"w{i}",
                                    name=f"w{i}_{k}")
                    nc.vector.tensor_tensor(
                        wt[:], ws[:], z_rep[:, o : o + wd], AL.subtract
                    )
                    wtiles[(i, k)] = wt

            def epilogue(i, mt, ps, row):
                o, wd = N_CHUNKS[i]
                ob = opool.tile([P, wd], dt.float32, tag=f"ob{i}", name=f"ob{i}")
                nc.vector.tensor_tensor(
                    ob[:], ps[:], b_rep[:, o : o + wd], AL.add
                )
                nc.scalar.dma_start(out=out_d[row : row + P, o : o + wd], in_=ob[:])

            # ---- phase 0: rows 0:1024, 8 psum banks, k-outer per group ----
            def phase0_group(i, xts0, xts1):
                o, wd = N_CHUNKS[i]
                pss = [
                    pspool.tile([P, wd], dt.float32, tag=f"ps{m}", name=f"ps{m}")
                    for m in range(8)
                ]
                for k in range(KT):
                    for mt in range(8):
                        xt = xts0[k] if mt < 4 else xts1[k]
                        nc.tensor.matmul(
                            pss[mt][:],
                            xt[:, (mt % 4) * P : (mt % 4 + 1) * P],
                            wtiles[(i, k)][:],
                            start=(k == 0),
                            stop=(k == KT - 1),
                        )
                for mt in range(8):
                    epilogue(i, mt, pss[mt], mt * P)

            # ---- steady chunk: one 512-row unit, 4 psum banks ----
            def steady_chunk(u, xts):
                base = 4 * (u % 2)
                for i, (o, wd) in enumerate(N_CHUNKS):
                    pss = [
                        pspool.tile(
                            [P, wd], dt.float32,
                            tag=f"ps{base + m}", name=f"ps{base + m}",
                        )
                        for m in range(4)
                    ]
                    for k in range(KT):
                        for mt in range(4):
                            nc.tensor.matmul(
                                pss[mt][:],
                                xts[k][:, mt * P : (mt + 1) * P],
                                wtiles[(i, k)][:],
                                start=(k == 0),
                                stop=(k == KT - 1),
                            )
                    for mt in range(4):
                        epilogue(i, mt, pss[mt], u * MU + mt * P)

            # ---- program ----
            cast_unit(0)
            cast_unit(1)
            nc.gpsimd.dma_start(out=b_rep[:], in_=b_d[None, :].to_broadcast([P, NSH]))
            xts0 = transpose_unit(0)
            xts1 = transpose_unit(1)

            unpack_group(0)
            phase0_group(0, xts0, xts1)
            unpack_group(1)
            if n_units > 2:
                cast_unit(2)
            phase0_group(1, xts0, xts1)
            unpack_group(2)
            if n_units > 3:
                cast_unit(3)
            xts_next = transpose_unit(2) if n_units > 2 else None
            phase0_group(2, xts0, xts1)

            for u in range(2, n_units):
                xts = xts_next
                if u + 2 < n_units:
                    cast_unit(u + 2)
                if u + 1 < n_units:
                    xts_next = transpose_unit(u + 1)
                steady_chunk(u, xts)

    nc.compile()
    return nc


_SHAMT = (4 * (np.arange(P, dtype=np.int32) % 8)).reshape(P, 1)


def make_in_maps(x2d, qweight, scales, zeros, bias):
    """Per-core input maps (host-side sharding / layout prep only)."""
    in_maps = []
    for c in range(NCORES):
        sl = slice(c * NSH, (c + 1) * NSH)
        in_maps.append(
            {
                "x": x2d,
                "qw": np.ascontiguousarray(
                    np.repeat(qweight[:, sl], 8, axis=0)
                ),
                "scales": np.ascontiguousarray(scales[sl, 0]),
                "zeros": np.ascontiguousarray(zeros[sl, 0]),
                "bias": np.ascontiguousarray(bias[sl]),
                "shamt": _SHAMT,
            }
        )
    return in_maps


_NC_CACHE = {}


def _get_nc(m_rows):
    if m_rows not in _NC_CACHE:
        _NC_CACHE[m_rows] = build(m_rows)
    return _NC_CACHE[m_rows]


def run_spmd(x2d, qweight, scales, zeros, bias, trace=False, **kwargs):
    """Run on the 8 NeuronCores; returns (out2d [8192, 11008] f32, results)."""
    from concourse.bass_utils import run_bass_kernel_spmd

    m_rows = x2d.shape[0]
    nc = _get_nc(m_rows)
    in_maps = make_in_maps(x2d, qweight, scales, zeros, bias)
    res = run_bass_kernel_spmd(
        nc, in_maps, list(range(NCORES)), trace=trace, **kwargs
    )
    outs = [res.results[c]["out"] for c in range(NCORES)]
    out2d = np.concatenate(outs, axis=1)
    return out2d, res


def kernel(x, qweight, scales, zeros, bias):
    x = np.asarray(x, dtype=np.float32)
    qweight = np.asarray(qweight, dtype=np.int32)
    scales = np.asarray(scales, dtype=np.float32)
    zeros = np.asarray(zeros, dtype=np.float32)
    bias = np.asarray(bias, dtype=np.float32)

    b, s, k_in = x.shape
    x2d = np.ascontiguousarray(x.reshape(b * s, k_in))
    out2d, _ = run_spmd(x2d, qweight, scales, zeros, bias)
    return out2d.reshape(b, s, OUT)


# revision 30
# speedup vs baseline: 1.2850x; 1.2022x over previous
"""Trainium2 Bass kernel for nn_Autograd4bitQuantLinear (4-bit quant linear).

Computes out = x @ dequant4(qweight, scales, zeros) + bias where
  x:       (4, 2048, 4096) f32
  qweight: (512, 11008)    i32  (8 nibbles packed per int32 along rows)
  scales:  (11008, 1)      f32
  zeros:   (11008, 1)      f32
  bias:    (11008,)        f32
  out:     (4, 2048, 11008) f32

Strategy (tensor-parallel over 8 NeuronCores, column-sharded out_features):
  - Each core owns 1376 output columns; x is replicated.
  - W is kept as the RAW nibble values q in fp8e4 (exact for 0..15),
    SBUF-resident at 43KB/partition (vs 86KB for dequantized bf16).
    The PE computes x_bf16 @ q_fp8 (mixed-dtype matmul, HW-verified) and
    the affine dequant folds into the epilogue:
        out = (x @ q) * s - rowsum(x) * z + b
    rowsum(x) comes for free as a 353rd ones-column in PSUM group 2, so
    groups are processed in order (2, 0, 1) and the rowsum is copied to
    SBUF before groups 0/1 retire.
  - The fp8 W residency is what lets the x-tile pool hold 58 [128, 1024]
    bf16 tiles: whole-chunk double buffering with only 32 transposes per
    1024 rows. Transpose issue cost (~1.4us + ~2.7us semaphore handling,
    serial per queue sequencer) was the binding resource of every
    earlier version; here it is 32 per chunk split across BOTH HWDGE
    queues (sync + scalar; concurrency HW-verified) = ~69us/queue per
    147us chunk.
  - x staging: 8 column-slice cast DMAs (f32->bf16, SWDGE) per chunk
    into DRAM scratch, issued two chunks ahead and before any paced
    loads (anti-convoy), then the 32 xbar transposes.
  - PE: chunks of 1024 rows, group-major k-outer, mt-inner over all 8
    PSUM banks (keeps per-k x-tile consumption at ~1.7us, matching
    unpack and transpose supply).
  - Queue roles: sync/scalar = transposes (split by k parity); gpsimd
    (SWDGE) = casts, qweight loads, output stores, broadcasts.
  - Nibble unpack (one-time, JIT before each chunk-0 group pass):
    STT shift/and -> int32, tensor_copy -> fp8 (exact, HW-verified).
"""

import sys

sys.path.insert(0, "/opt/trn_rl_repo")

import numpy as np

import concourse.bass as bass
import concourse.mybir as mybir
from concourse import bacc
from concourse.tile import TileContext
from concourse.tile_rust import add_dep_helper


dt = mybir.dt
AL = mybir.AluOpType

P = 128
IN = 4096  # contraction dim (in_features)
OUT = 11008  # out_features
M_ROWS = 8192  # 4 * 2048
NCORES = 8
NSH = OUT // NCORES  # 1376 output columns per core
KT = IN // P  # 32 k-tiles
MC = 1024  # rows per chunk
# psum n-chunks; group 2 carries a ones-column for rowsum(x) (352+1)
N_CHUNKS = ((0, 512), (512, 512), (1024, 352))
GROUP_ORDER = (2, 0, 1)  # rowsum group first
XT_BUFS = 56  # [128, 1024] bf16 tiles; chunk uses 32, next chunk prefetches


def build(m_rows=M_ROWS, debug=False):
    """Build + compile the single-core Tile program (SPMD: same on all cores)."""
    assert m_rows % MC == 0
    nc = bacc.Bacc(None, target_bir_lowering=False, debug=debug)

    x_d = nc.dram_tensor("x", [m_rows, IN], dt.float32, kind="ExternalInput")
    qw_d = nc.dram_tensor("qw", [IN, NSH], dt.int32, kind="ExternalInput")
    s_d = nc.dram_tensor("scales", [NSH], dt.float32, kind="ExternalInput")
    z_d = nc.dram_tensor("zeros", [NSH], dt.float32, kind="ExternalInput")
    b_d = nc.dram_tensor("bias", [NSH], dt.float32, kind="ExternalInput")
    shamt_d = nc.dram_tensor("shamt", [P, 1], dt.int32, kind="ExternalInput")
    out_d = nc.dram_tensor("out", [m_rows, NSH], dt.float32, kind="ExternalOutput")

    n_chunks = m_rows // MC
    mt_per_chunk = MC // P  # 8

    with TileContext(nc) as tc:
        with (
            tc.tile_pool(name="singles", bufs=1) as singles,
            tc.tile_pool(name="w", bufs=KT) as wpool,
            tc.tile_pool(name="unpack", bufs=3) as upool,
            tc.tile_pool(name="xbf", bufs=2, space="DRAM") as xbfpool,
            tc.tile_pool(name="xt", bufs=XT_BUFS) as xtpool,
            tc.tile_pool(name="osb", bufs=2) as opool,
            tc.tile_pool(name="rs", bufs=2) as rspool,
            tc.tile_pool(name="ps", bufs=1, space="PSUM") as pspool,
        ):
            # ---- constants ----
            s_rep = singles.tile([P, NSH], dt.float32, tag="s_rep")
            nc.gpsimd.dma_start(out=s_rep[:], in_=s_d[None, :].to_broadcast([P, NSH]))
            zt = singles.tile([P, NSH], dt.float32, tag="zt")
            nc.gpsimd.dma_start(out=zt[:], in_=z_d[None, :].to_broadcast([P, NSH]))
            shamt = singles.tile([P, 1], dt.int32, tag="shamt")
            nc.scalar.dma_start(out=shamt[:], in_=shamt_d[:])
            mask = singles.tile([P, 1], dt.int32, tag="mask")
            nc.vector.memset(mask[:], 15)
            negz = singles.tile([P, NSH], dt.float32, tag="negz")
            nc.scalar.mul(negz[:], zt[:], -1.0)
            b_rep = singles.tile([P, NSH], dt.float32, tag="b_rep")

            last_xpose = {}
            xbf_slices = {}

            def cast_chunk(c):
                """8 column-slice cast DMAs per chunk (parallel rings,
                bounded head-of-line delay)."""
                r0 = c * MC
                slices = []
                for j in range(8):
                    t = xbfpool.tile([MC, 512], dt.bfloat16, tag=f"xbf_{j}",
                                     name=f"xbf{c}_{j}")
                    ci = nc.gpsimd.dma_start(
                        out=t[:], in_=x_d[r0 : r0 + MC, j * 512 : (j + 1) * 512]
                    )
                    if c - 2 in last_xpose:
                        add_dep_helper(
                            ci.ins,
                            last_xpose[c - 2].ins,
                            sync=True,
                            reason="throttle x cast chain",
                        )
                    slices.append(t)
                xbf_slices[c] = slices

            def transpose_chunk(c):
                """32 xbar transposes -> [128, 1024] tiles, sync queue ONLY.
                (Concurrent transposes from both HWDGE queues corrupt data
                under load - the xbar appears to be a shared block; verified
                twice on HW. With 24 spare xt buffers the waits are pre-met
                and the sync sequencer sustains ~2us per transpose.)"""
                xts = []
                for k in range(KT):
                    xt = xtpool.tile([P, MC], dt.bfloat16, tag="xt", name="xt")
                    j = k // 4
                    src = xbf_slices[c][j][
                        :, k * 128 - j * 512 : (k + 1) * 128 - j * 512
                    ]
                    ti = nc.sync.dma_start(out=xt[:], in_=src, transpose=True)
                    xts.append(xt)
                last_xpose[c] = ti
                return xts

            # ---- W unpack to fp8 (JIT for chunk 0, resident after) ----
            wtiles = {}  # (i, k) -> [P, wd(+1)] fp8 tile

            def unpack_group(i):
                o, wd = N_CHUNKS[i]
                ones_col = 1 if i == 2 else 0
                qts = []
                for k in range(KT):
                    qt = upool.tile([P, wd], dt.int32, tag="qt", name="qt")
                    nc.gpsimd.dma_start(
                        out=qt[:], in_=qw_d[k * P : (k + 1) * P, o : o + wd]
                    )
                    qts.append(qt)
                for k in range(KT):
                    nib = upool.tile([P, wd], dt.int32, tag="nib", name="nib",
                                     bufs=1)
                    nc.vector.scalar_tensor_tensor(
                        nib[:],
                        qts[k][:],
                        shamt[:, 0:1],
                        mask[:, 0:1].to_broadcast([P, wd]),
                        AL.logical_shift_right,
                        AL.bitwise_and,
                    )
                    wt = wpool.tile([P, wd + ones_col], dt.float8e4,
                                    tag=f"w{i}", name=f"w{i}_{k}")
                    nc.vector.tensor_copy(wt[:, 0:wd], nib[:])
                    if ones_col:
                        nc.vector.memset(wt[:, wd : wd + 1], 1)
                    wtiles[(i, k)] = wt

            # ---- epilogue: out = (x@q)*s + rowsum(x)*(-z) + b ----
            rs_tiles = {}

            def epilogue(i, mt, ps, row):
                o, wd = N_CHUNKS[i]
                if i == 2:
                    rs = rspool.tile([P, 1], dt.float32, tag=f"rs{mt}",
                                     name=f"rs{mt}")
                    nc.vector.tensor_copy(rs[:], ps[:, wd : wd + 1])
                    rs_tiles[mt] = rs
                et = opool.tile([P, wd], dt.float32, tag=f"et{i}", name="et",
                                bufs=1)
                nc.vector.tensor_tensor(
                    et[:], ps[:, 0:wd], s_rep[:, o : o + wd], AL.mult
                )
                et2 = opool.tile([P, wd], dt.float32, tag=f"e2{i}", name="e2",
                                 bufs=1)
                nc.vector.scalar_tensor_tensor(
                    et2[:],
                    negz[:, o : o + wd],
                    rs_tiles[mt][:, 0:1],
                    et[:],
                    AL.mult,
                    AL.add,
                )
                ob = opool.tile([P, wd], dt.float32, tag=f"ob{i}", name=f"ob{i}")
                nc.vector.tensor_tensor(
                    ob[:], et2[:], b_rep[:, o : o + wd], AL.add
                )
                nc.gpsimd.dma_start(out=out_d[row : row + P, o : o + wd], in_=ob[:])

            def chunk_group(c, i, xts):
                o, wd = N_CHUNKS[i]
                ones_col = 1 if i == 2 else 0
                pss = [
                    pspool.tile([P, wd + ones_col], dt.float32,
                                tag=f"ps{m}", name=f"ps{m}")
                    for m in range(mt_per_chunk)
                ]
                for k in range(KT):
                    for mt in range(mt_per_chunk):
                        nc.tensor.matmul(
                            pss[mt][:],
                            xts[k][:, mt * P : (mt + 1) * P],
                            wtiles[(i, k)][:],
                            start=(k == 0),
                            stop=(k == KT - 1),
                        )
                for mt in range(mt_per_chunk):
                    epilogue(i, mt, pss[mt], c * MC + mt * P)

            # ---- program ----
            cast_chunk(0)
            nc.gpsimd.dma_start(out=b_rep[:], in_=b_d[None, :].to_broadcast([P, NSH]))
            if n_chunks > 1:
                cast_chunk(1)
            xts_cur = transpose_chunk(0)

            xts_next = None
            for gi, i in enumerate(GROUP_ORDER):
                unpack_group(i)
                chunk_group(0, i, xts_cur)
                if gi == 1:
                    if n_chunks > 2:
                        cast_chunk(2)
                    if n_chunks > 1:
                        xts_next = transpose_chunk(1)

            for c in range(1, n_chunks):
                xts = xts_next
                if c + 2 < n_chunks:
                    cast_chunk(c + 2)
                chunk_group(c, GROUP_ORDER[0], xts)
                chunk_group(c, GROUP_ORDER[1], xts)
                if c + 1 < n_chunks:
                    xts_next = transpose_chunk(c + 1)
                chunk_group(c, GROUP_ORDER[2], xts)

    nc.compile()
    return nc


_SHAMT = (4 * (np.arange(P, dtype=np.int32) % 8)).reshape(P, 1)


def make_in_maps(x2d, qweight, scales, zeros, bias):
    """Per-core input maps (host-side sharding / layout prep only)."""
    in_maps = []
    for c in range(NCORES):
        sl = slice(c * NSH, (c + 1) * NSH)
        in_maps.append(
            {
                "x": x2d,
                "qw": np.ascontiguousarray(
                    np.repeat(qweight[:, sl], 8, axis=0)
                ),
                "scales": np.ascontiguousarray(scales[sl, 0]),
                "zeros": np.ascontiguousarray(zeros[sl, 0]),
                "bias": np.ascontiguousarray(bias[sl]),
                "shamt": _SHAMT,
            }
        )
    return in_maps


_NC_CACHE = {}


def _get_nc(m_rows):
    if m_rows not in _NC_CACHE:
        _NC_CACHE[m_rows] = build(m_rows)
    return _NC_CACHE[m_rows]


def run_spmd(x2d, qweight, scales, zeros, bias, trace=False, **kwargs):
    """Run on the 8 NeuronCores; returns (out2d [8192, 11008] f32, results)."""
    from concourse.bass_utils import run_bass_kernel_spmd

    m_rows = x2d.shape[0]
    nc = _get_nc(m_rows)
    in_maps = make_in_maps(x2d, qweight, scales, zeros, bias)
    res = run_bass_kernel_spmd(
        nc, in_maps, list(range(NCORES)), trace=trace, **kwargs
    )
    outs = [res.results[c]["out"] for c in range(NCORES)]
    out2d = np.concatenate(outs, axis=1)
    return out2d, res


def kernel(x, qweight, scales, zeros, bias):
    x = np.asarray(x, dtype=np.float32)
    qweight = np.asarray(qweight, dtype=np.int32)
    scales = np.asarray(scales, dtype=np.float32)
    zeros = np.asarray(zeros, dtype=np.float32)
    bias = np.asarray(bias, dtype=np.float32)

    b, s, k_in = x.shape
    x2d = np.ascontiguousarray(x.reshape(b * s, k_in))
    out2d, _ = run_spmd(x2d, qweight, scales, zeros, bias)
    return out2d.reshape(b, s, OUT)


# revision 31
# speedup vs baseline: 1.3029x; 1.0139x over previous
"""Trainium2 Bass kernel for nn_Autograd4bitQuantLinear (4-bit quant linear).

Computes out = x @ dequant4(qweight, scales, zeros) + bias where
  x:       (4, 2048, 4096) f32
  qweight: (512, 11008)    i32  (8 nibbles packed per int32 along rows)
  scales:  (11008, 1)      f32
  zeros:   (11008, 1)      f32
  bias:    (11008,)        f32
  out:     (4, 2048, 11008) f32

Strategy (tensor-parallel over 8 NeuronCores, column-sharded out_features):
  - Each core owns 1376 output columns; x is replicated.
  - W is kept as the RAW nibble values q in fp8e4 (exact for 0..15),
    SBUF-resident at 43KB/partition (vs 86KB for dequantized bf16).
    The PE computes x_bf16 @ q_fp8 (mixed-dtype matmul, HW-verified) and
    the affine dequant folds into the epilogue:
        out = (x @ q) * s - rowsum(x) * z + b
    rowsum(x) comes for free as a 353rd ones-column in PSUM group 2, so
    groups are processed in order (2, 0, 1) and the rowsum is copied to
    SBUF before groups 0/1 retire.
  - The fp8 W residency is what lets the x-tile pool hold 58 [128, 1024]
    bf16 tiles: whole-chunk double buffering with only 32 transposes per
    1024 rows. Transpose issue cost (~1.4us + ~2.7us semaphore handling,
    serial per queue sequencer) was the binding resource of every
    earlier version; here it is 32 per chunk split across BOTH HWDGE
    queues (sync + scalar; concurrency HW-verified) = ~69us/queue per
    147us chunk.
  - x staging: 8 column-slice cast DMAs (f32->bf16, SWDGE) per chunk
    into DRAM scratch, issued two chunks ahead and before any paced
    loads (anti-convoy), then the 32 xbar transposes.
  - PE: chunks of 1024 rows, group-major k-outer, mt-inner over all 8
    PSUM banks (keeps per-k x-tile consumption at ~1.7us, matching
    unpack and transpose supply).
  - Queue roles: sync/scalar = transposes (split by k parity); gpsimd
    (SWDGE) = casts, qweight loads, output stores, broadcasts.
  - Nibble unpack (one-time, JIT before each chunk-0 group pass):
    STT shift/and -> int32, tensor_copy -> fp8 (exact, HW-verified).
"""

import sys

sys.path.insert(0, "/opt/trn_rl_repo")

import numpy as np

import concourse.bass as bass
import concourse.mybir as mybir
from concourse import bacc
from concourse.tile import TileContext
from concourse.tile_rust import add_dep_helper


dt = mybir.dt
AL = mybir.AluOpType

P = 128
IN = 4096  # contraction dim (in_features)
OUT = 11008  # out_features
M_ROWS = 8192  # 4 * 2048
NCORES = 8
NSH = OUT // NCORES  # 1376 output columns per core
KT = IN // P  # 32 k-tiles
MC = 1024  # rows per chunk
# psum n-chunks; group 2 carries a ones-column for rowsum(x) (352+1)
N_CHUNKS = ((0, 512), (512, 512), (1024, 352))
GROUP_ORDER = (2, 0, 1)  # rowsum group first
XT_BUFS = 56  # [128, 1024] bf16 tiles; chunk uses 32, next chunk prefetches


def build(m_rows=M_ROWS, debug=False):
    """Build + compile the single-core Tile program (SPMD: same on all cores)."""
    assert m_rows % MC == 0
    nc = bacc.Bacc(None, target_bir_lowering=False, debug=debug)

    x_d = nc.dram_tensor("x", [m_rows, IN], dt.float32, kind="ExternalInput")
    qw_d = nc.dram_tensor("qw", [IN, NSH], dt.int32, kind="ExternalInput")
    s_d = nc.dram_tensor("scales", [NSH], dt.float32, kind="ExternalInput")
    z_d = nc.dram_tensor("zeros", [NSH], dt.float32, kind="ExternalInput")
    b_d = nc.dram_tensor("bias", [NSH], dt.float32, kind="ExternalInput")
    shamt_d = nc.dram_tensor("shamt", [P, 1], dt.int32, kind="ExternalInput")
    out_d = nc.dram_tensor("out", [m_rows, NSH], dt.float32, kind="ExternalOutput")

    n_chunks = m_rows // MC
    mt_per_chunk = MC // P  # 8

    with TileContext(nc) as tc:
        with (
            tc.tile_pool(name="singles", bufs=1) as singles,
            tc.tile_pool(name="w", bufs=KT) as wpool,
            tc.tile_pool(name="unpack", bufs=3) as upool,
            tc.tile_pool(name="xbf", bufs=2, space="DRAM") as xbfpool,
            tc.tile_pool(name="xt", bufs=XT_BUFS) as xtpool,
            tc.tile_pool(name="osb", bufs=2) as opool,
            tc.tile_pool(name="rs", bufs=2) as rspool,
            tc.tile_pool(name="ps", bufs=1, space="PSUM") as pspool,
        ):
            # ---- constants ----
            s_rep = singles.tile([P, NSH], dt.float32, tag="s_rep")
            nc.gpsimd.dma_start(out=s_rep[:], in_=s_d[None, :].to_broadcast([P, NSH]))
            zt = singles.tile([P, NSH], dt.float32, tag="zt")
            nc.gpsimd.dma_start(out=zt[:], in_=z_d[None, :].to_broadcast([P, NSH]))
            shamt = singles.tile([P, 1], dt.int32, tag="shamt")
            nc.scalar.dma_start(out=shamt[:], in_=shamt_d[:])
            mask = singles.tile([P, 1], dt.int32, tag="mask")
            nc.vector.memset(mask[:], 15)
            negz = singles.tile([P, NSH], dt.float32, tag="negz")
            nc.scalar.mul(negz[:], zt[:], -1.0)
            b_rep = singles.tile([P, NSH], dt.float32, tag="b_rep")

            last_xpose = {}
            xbf_slices = {}

            def cast_chunk(c):
                """8 column-slice cast DMAs per chunk (parallel rings,
                bounded head-of-line delay)."""
                r0 = c * MC
                slices = []
                for j in range(8):
                    t = xbfpool.tile([MC, 512], dt.bfloat16, tag=f"xbf_{j}",
                                     name=f"xbf{c}_{j}")
                    ci = nc.gpsimd.dma_start(
                        out=t[:], in_=x_d[r0 : r0 + MC, j * 512 : (j + 1) * 512]
                    )
                    if c - 2 in last_xpose:
                        add_dep_helper(
                            ci.ins,
                            last_xpose[c - 2].ins,
                            sync=True,
                            reason="throttle x cast chain",
                        )
                    slices.append(t)
                xbf_slices[c] = slices

            def transpose_chunk(c):
                """32 xbar transposes -> [128, 1024] tiles, sync queue ONLY.
                (Concurrent transposes from both HWDGE queues corrupt data
                under load - the xbar appears to be a shared block; verified
                twice on HW. With 24 spare xt buffers the waits are pre-met
                and the sync sequencer sustains ~2us per transpose.)"""
                xts = []
                for k in range(KT):
                    xt = xtpool.tile([P, MC], dt.bfloat16, tag="xt", name="xt")
                    j = k // 4
                    src = xbf_slices[c][j][
                        :, k * 128 - j * 512 : (k + 1) * 128 - j * 512
                    ]
                    ti = nc.sync.dma_start(out=xt[:], in_=src, transpose=True)
                    xts.append(xt)
                last_xpose[c] = ti
                return xts

            # ---- W unpack to fp8 (JIT for chunk 0, resident after) ----
            wtiles = {}  # (i, k) -> [P, wd(+1)] fp8 tile

            def unpack_group(i):
                o, wd = N_CHUNKS[i]
                ones_col = 1 if i == 2 else 0
                qts = []
                for k in range(KT):
                    qt = upool.tile([P, wd], dt.int32, tag="qt", name="qt")
                    nc.gpsimd.dma_start(
                        out=qt[:], in_=qw_d[k * P : (k + 1) * P, o : o + wd]
                    )
                    qts.append(qt)
                for k in range(KT):
                    nib = upool.tile([P, wd], dt.int32, tag="nib", name="nib",
                                     bufs=1)
                    nc.vector.scalar_tensor_tensor(
                        nib[:],
                        qts[k][:],
                        shamt[:, 0:1],
                        mask[:, 0:1].to_broadcast([P, wd]),
                        AL.logical_shift_right,
                        AL.bitwise_and,
                    )
                    wt = wpool.tile([P, wd + ones_col], dt.float8e4,
                                    tag=f"w{i}", name=f"w{i}_{k}")
                    nc.vector.tensor_copy(wt[:, 0:wd], nib[:])
                    if ones_col:
                        nc.vector.memset(wt[:, wd : wd + 1], 1)
                    wtiles[(i, k)] = wt

            # ---- epilogue: out = (x@q)*s + rowsum(x)*(-z) + b ----
            rs_tiles = {}

            def epilogue(i, mt, ps, row):
                o, wd = N_CHUNKS[i]
                if i == 2:
                    rs = rspool.tile([P, 1], dt.float32, tag=f"rs{mt}",
                                     name=f"rs{mt}")
                    nc.vector.tensor_copy(rs[:], ps[:, wd : wd + 1])
                    rs_tiles[mt] = rs
                et = opool.tile([P, wd], dt.float32, tag=f"et{i}", name="et",
                                bufs=1)
                nc.vector.tensor_tensor(
                    et[:], ps[:, 0:wd], s_rep[:, o : o + wd], AL.mult
                )
                et2 = opool.tile([P, wd], dt.float32, tag=f"e2{i}", name="e2",
                                 bufs=1)
                nc.vector.scalar_tensor_tensor(
                    et2[:],
                    negz[:, o : o + wd],
                    rs_tiles[mt][:, 0:1],
                    et[:],
                    AL.mult,
                    AL.add,
                )
                ob = opool.tile([P, wd], dt.float32, tag=f"ob{i}", name=f"ob{i}")
                nc.vector.tensor_tensor(
                    ob[:], et2[:], b_rep[:, o : o + wd], AL.add
                )
                # stores on scalar: gpsimd stores pace on the epilogues and
                # would convoy the next chunk's cast slices behind them
                nc.scalar.dma_start(
                    out=out_d[row : row + P, o : o + wd], in_=ob[:]
                )

            def chunk_group(c, i, xts):
                o, wd = N_CHUNKS[i]
                ones_col = 1 if i == 2 else 0
                pss = [
                    pspool.tile([P, wd + ones_col], dt.float32,
                                tag=f"ps{m}", name=f"ps{m}")
                    for m in range(mt_per_chunk)
                ]
                for k in range(KT):
                    for mt in range(mt_per_chunk):
                        nc.tensor.matmul(
                            pss[mt][:],
                            xts[k][:, mt * P : (mt + 1) * P],
                            wtiles[(i, k)][:],
                            start=(k == 0),
                            stop=(k == KT - 1),
                        )
                for mt in range(mt_per_chunk):
                    epilogue(i, mt, pss[mt], c * MC + mt * P)

            # ---- program ----
            cast_chunk(0)
            nc.gpsimd.dma_start(out=b_rep[:], in_=b_d[None, :].to_broadcast([P, NSH]))
            if n_chunks > 1:
                cast_chunk(1)
            xts_cur = transpose_chunk(0)

            xts_next = None
            for gi, i in enumerate(GROUP_ORDER):
                unpack_group(i)
                chunk_group(0, i, xts_cur)
                if gi == 1:
                    if n_chunks > 2:
                        cast_chunk(2)
                    if n_chunks > 1:
                        xts_next = transpose_chunk(1)

            for c in range(1, n_chunks):
                xts = xts_next
                if c + 2 < n_chunks:
                    cast_chunk(c + 2)
                chunk_group(c, GROUP_ORDER[0], xts)
                chunk_group(c, GROUP_ORDER[1], xts)
                if c + 1 < n_chunks:
                    xts_next = transpose_chunk(c + 1)
                chunk_group(c, GROUP_ORDER[2], xts)

    nc.compile()
    return nc


_SHAMT = (4 * (np.arange(P, dtype=np.int32) % 8)).reshape(P, 1)


def make_in_maps(x2d, qweight, scales, zeros, bias):
    """Per-core input maps (host-side sharding / layout prep only)."""
    in_maps = []
    for c in range(NCORES):
        sl = slice(c * NSH, (c + 1) * NSH)
        in_maps.append(
            {
                "x": x2d,
                "qw": np.ascontiguousarray(
                    np.repeat(qweight[:, sl], 8, axis=0)
                ),
                "scales": np.ascontiguousarray(scales[sl, 0]),
                "zeros": np.ascontiguousarray(zeros[sl, 0]),
                "bias": np.ascontiguousarray(bias[sl]),
                "shamt": _SHAMT,
            }
        )
    return in_maps


_NC_CACHE = {}


def _get_nc(m_rows):
    if m_rows not in _NC_CACHE:
        _NC_CACHE[m_rows] = build(m_rows)
    return _NC_CACHE[m_rows]


def run_spmd(x2d, qweight, scales, zeros, bias, trace=False, **kwargs):
    """Run on the 8 NeuronCores; returns (out2d [8192, 11008] f32, results)."""
    from concourse.bass_utils import run_bass_kernel_spmd

    m_rows = x2d.shape[0]
    nc = _get_nc(m_rows)
    in_maps = make_in_maps(x2d, qweight, scales, zeros, bias)
    res = run_bass_kernel_spmd(
        nc, in_maps, list(range(NCORES)), trace=trace, **kwargs
    )
    outs = [res.results[c]["out"] for c in range(NCORES)]
    out2d = np.concatenate(outs, axis=1)
    return out2d, res


def kernel(x, qweight, scales, zeros, bias):
    x = np.asarray(x, dtype=np.float32)
    qweight = np.asarray(qweight, dtype=np.int32)
    scales = np.asarray(scales, dtype=np.float32)
    zeros = np.asarray(zeros, dtype=np.float32)
    bias = np.asarray(bias, dtype=np.float32)

    b, s, k_in = x.shape
    x2d = np.ascontiguousarray(x.reshape(b * s, k_in))
    out2d, _ = run_spmd(x2d, qweight, scales, zeros, bias)
    return out2d.reshape(b, s, OUT)


# revision 32
# speedup vs baseline: 1.3725x; 1.0535x over previous
"""Trainium2 Bass kernel for nn_Autograd4bitQuantLinear (4-bit quant linear).

Computes out = x @ dequant4(qweight, scales, zeros) + bias where
  x:       (4, 2048, 4096) f32
  qweight: (512, 11008)    i32  (8 nibbles packed per int32 along rows)
  scales:  (11008, 1)      f32
  zeros:   (11008, 1)      f32
  bias:    (11008,)        f32
  out:     (4, 2048, 11008) f32

Strategy (tensor-parallel over 8 NeuronCores, column-sharded out_features):
  - Each core owns 1376 output columns; x is replicated.
  - W is kept as the RAW nibble values q in fp8e4 (exact for 0..15),
    SBUF-resident at 43KB/partition (vs 86KB for dequantized bf16).
    The PE computes x_bf16 @ q_fp8 (mixed-dtype matmul, HW-verified) and
    the affine dequant folds into the epilogue:
        out = (x @ q) * s - rowsum(x) * z + b
    rowsum(x) comes for free as a 353rd ones-column in PSUM group 2, so
    groups are processed in order (2, 0, 1) and the rowsum is copied to
    SBUF before groups 0/1 retire.
  - The fp8 W residency is what lets the x-tile pool hold 58 [128, 1024]
    bf16 tiles: whole-chunk double buffering with only 32 transposes per
    1024 rows. Transpose issue cost (~1.4us + ~2.7us semaphore handling,
    serial per queue sequencer) was the binding resource of every
    earlier version; here it is 32 per chunk split across BOTH HWDGE
    queues (sync + scalar; concurrency HW-verified) = ~69us/queue per
    147us chunk.
  - x staging: 8 column-slice cast DMAs (f32->bf16, SWDGE) per chunk
    into DRAM scratch, issued two chunks ahead and before any paced
    loads (anti-convoy), then the 32 xbar transposes.
  - PE: chunks of 1024 rows, group-major k-outer, mt-inner over all 8
    PSUM banks (keeps per-k x-tile consumption at ~1.7us, matching
    unpack and transpose supply).
  - Queue roles: sync/scalar = transposes (split by k parity); gpsimd
    (SWDGE) = casts, qweight loads, output stores, broadcasts.
  - Nibble unpack (one-time, JIT before each chunk-0 group pass):
    STT shift/and -> int32, tensor_copy -> fp8 (exact, HW-verified).
"""

import sys

sys.path.insert(0, "/opt/trn_rl_repo")

import numpy as np

import concourse.bass as bass
import concourse.mybir as mybir
from concourse import bacc
from concourse.tile import TileContext
from concourse.tile_rust import add_dep_helper


dt = mybir.dt
AL = mybir.AluOpType

P = 128
IN = 4096  # contraction dim (in_features)
OUT = 11008  # out_features
M_ROWS = 8192  # 4 * 2048
NCORES = 8
NSH = OUT // NCORES  # 1376 output columns per core
KT = IN // P  # 32 k-tiles
MC = 1024  # rows per chunk
# psum n-chunks; group 2 carries a ones-column for rowsum(x) (352+1)
N_CHUNKS = ((0, 512), (512, 512), (1024, 352))
GROUP_ORDER = (2, 0, 1)  # rowsum group first
XT_BUFS = 56  # [128, 1024] bf16 tiles; chunk uses 32, next chunk prefetches


def build(m_rows=M_ROWS, debug=False):
    """Build + compile the single-core Tile program (SPMD: same on all cores)."""
    assert m_rows % MC == 0
    nc = bacc.Bacc(None, target_bir_lowering=False, debug=debug)

    x_d = nc.dram_tensor("x", [m_rows, IN], dt.float32, kind="ExternalInput")
    qw_d = nc.dram_tensor("qw", [IN, NSH], dt.int32, kind="ExternalInput")
    s_d = nc.dram_tensor("scales", [NSH], dt.float32, kind="ExternalInput")
    z_d = nc.dram_tensor("zeros", [NSH], dt.float32, kind="ExternalInput")
    b_d = nc.dram_tensor("bias", [NSH], dt.float32, kind="ExternalInput")
    shamt_d = nc.dram_tensor("shamt", [P, 1], dt.int32, kind="ExternalInput")
    out_d = nc.dram_tensor("out", [m_rows, NSH], dt.float32, kind="ExternalOutput")

    n_chunks = m_rows // MC
    mt_per_chunk = MC // P  # 8

    with TileContext(nc) as tc:
        with (
            tc.tile_pool(name="singles", bufs=1) as singles,
            tc.tile_pool(name="w", bufs=KT) as wpool,
            tc.tile_pool(name="unpack", bufs=3) as upool,
            tc.tile_pool(name="xbf", bufs=4, space="DRAM") as xbfpool,
            tc.tile_pool(name="xt", bufs=XT_BUFS) as xtpool,
            tc.tile_pool(name="osb", bufs=2) as opool,
            tc.tile_pool(name="rs", bufs=2) as rspool,
            tc.tile_pool(name="ps", bufs=1, space="PSUM") as pspool,
        ):
            # ---- constants ----
            s_rep = singles.tile([P, NSH], dt.float32, tag="s_rep")
            nc.gpsimd.dma_start(out=s_rep[:], in_=s_d[None, :].to_broadcast([P, NSH]))
            zt = singles.tile([P, NSH], dt.float32, tag="zt")
            nc.gpsimd.dma_start(out=zt[:], in_=z_d[None, :].to_broadcast([P, NSH]))
            shamt = singles.tile([P, 1], dt.int32, tag="shamt")
            nc.scalar.dma_start(out=shamt[:], in_=shamt_d[:])
            mask = singles.tile([P, 1], dt.int32, tag="mask")
            nc.vector.memset(mask[:], 15)
            negz = singles.tile([P, NSH], dt.float32, tag="negz")
            nc.scalar.mul(negz[:], zt[:], -1.0)
            b_rep = singles.tile([P, NSH], dt.float32, tag="b_rep")

            last_xpose = {}
            xbf_slices = {}

            def cast_chunk(c):
                """8 column-slice cast DMAs per chunk (parallel rings,
                bounded head-of-line delay)."""
                r0 = c * MC
                slices = []
                for j in range(8):
                    t = xbfpool.tile([MC, 512], dt.bfloat16, tag=f"xbf_{j}",
                                     name=f"xbf{c}_{j}")
                    ci = nc.gpsimd.dma_start(
                        out=t[:], in_=x_d[r0 : r0 + MC, j * 512 : (j + 1) * 512]
                    )
                    if c - 2 in last_xpose:
                        add_dep_helper(
                            ci.ins,
                            last_xpose[c - 2].ins,
                            sync=True,
                            reason="throttle x cast chain",
                        )
                    slices.append(t)
                xbf_slices[c] = slices

            def transpose_chunk(c):
                """32 xbar transposes -> [128, 1024] tiles, sync queue ONLY.
                (Concurrent transposes from both HWDGE queues corrupt data
                under load - the xbar appears to be a shared block; verified
                twice on HW. With 24 spare xt buffers the waits are pre-met
                and the sync sequencer sustains ~2us per transpose.)"""
                xts = []
                for k in range(KT):
                    xt = xtpool.tile([P, MC], dt.bfloat16, tag="xt", name="xt")
                    j = k // 4
                    src = xbf_slices[c][j][
                        :, k * 128 - j * 512 : (k + 1) * 128 - j * 512
                    ]
                    ti = nc.sync.dma_start(out=xt[:], in_=src, transpose=True)
                    xts.append(xt)
                last_xpose[c] = ti
                return xts

            # ---- W unpack to fp8 (JIT for chunk 0, resident after) ----
            wtiles = {}  # (i, k) -> [P, wd(+1)] fp8 tile

            def unpack_group(i):
                o, wd = N_CHUNKS[i]
                ones_col = 1 if i == 2 else 0
                qts = []
                for k in range(KT):
                    qt = upool.tile([P, wd], dt.int32, tag="qt", name="qt")
                    nc.gpsimd.dma_start(
                        out=qt[:], in_=qw_d[k * P : (k + 1) * P, o : o + wd]
                    )
                    qts.append(qt)
                for k in range(KT):
                    nib = upool.tile([P, wd], dt.int32, tag="nib", name="nib",
                                     bufs=1)
                    nc.vector.scalar_tensor_tensor(
                        nib[:],
                        qts[k][:],
                        shamt[:, 0:1],
                        mask[:, 0:1].to_broadcast([P, wd]),
                        AL.logical_shift_right,
                        AL.bitwise_and,
                    )
                    wt = wpool.tile([P, wd + ones_col], dt.float8e4,
                                    tag=f"w{i}", name=f"w{i}_{k}")
                    nc.vector.tensor_copy(wt[:, 0:wd], nib[:])
                    if ones_col:
                        nc.vector.memset(wt[:, wd : wd + 1], 1)
                    wtiles[(i, k)] = wt

            # ---- epilogue: out = (x@q)*s + rowsum(x)*(-z) + b ----
            rs_tiles = {}

            def epilogue(i, mt, ps, row):
                o, wd = N_CHUNKS[i]
                if i == 2:
                    rs = rspool.tile([P, 1], dt.float32, tag=f"rs{mt}",
                                     name=f"rs{mt}")
                    nc.vector.tensor_copy(rs[:], ps[:, wd : wd + 1])
                    rs_tiles[mt] = rs
                et = opool.tile([P, wd], dt.float32, tag=f"et{i}", name="et",
                                bufs=1)
                nc.vector.tensor_tensor(
                    et[:], ps[:, 0:wd], s_rep[:, o : o + wd], AL.mult
                )
                et2 = opool.tile([P, wd], dt.float32, tag=f"e2{i}", name="e2",
                                 bufs=1)
                nc.vector.scalar_tensor_tensor(
                    et2[:],
                    negz[:, o : o + wd],
                    rs_tiles[mt][:, 0:1],
                    et[:],
                    AL.mult,
                    AL.add,
                )
                ob = opool.tile([P, wd], dt.float32, tag=f"ob{i}", name=f"ob{i}")
                nc.vector.tensor_tensor(
                    ob[:], et2[:], b_rep[:, o : o + wd], AL.add
                )
                # stores on scalar: gpsimd stores pace on the epilogues and
                # would convoy the next chunk's cast slices behind them
                nc.scalar.dma_start(
                    out=out_d[row : row + P, o : o + wd], in_=ob[:]
                )

            def chunk_group(c, i, xts):
                o, wd = N_CHUNKS[i]
                ones_col = 1 if i == 2 else 0
                pss = [
                    pspool.tile([P, wd + ones_col], dt.float32,
                                tag=f"ps{m}", name=f"ps{m}")
                    for m in range(mt_per_chunk)
                ]
                for k in range(KT):
                    for mt in range(mt_per_chunk):
                        nc.tensor.matmul(
                            pss[mt][:],
                            xts[k][:, mt * P : (mt + 1) * P],
                            wtiles[(i, k)][:],
                            start=(k == 0),
                            stop=(k == KT - 1),
                        )
                for mt in range(mt_per_chunk):
                    epilogue(i, mt, pss[mt], c * MC + mt * P)

            # ---- program ----
            cast_chunk(0)
            nc.gpsimd.dma_start(out=b_rep[:], in_=b_d[None, :].to_broadcast([P, NSH]))
            if n_chunks > 1:
                cast_chunk(1)
            xts_cur = transpose_chunk(0)

            xts_next = None
            for gi, i in enumerate(GROUP_ORDER):
                unpack_group(i)
                chunk_group(0, i, xts_cur)
                if gi == 1:
                    if n_chunks > 2:
                        cast_chunk(2)
                    if n_chunks > 1:
                        xts_next = transpose_chunk(1)

            for c in range(1, n_chunks):
                xts = xts_next
                if c + 2 < n_chunks:
                    cast_chunk(c + 2)
                chunk_group(c, GROUP_ORDER[0], xts)
                chunk_group(c, GROUP_ORDER[1], xts)
                if c + 1 < n_chunks:
                    xts_next = transpose_chunk(c + 1)
                chunk_group(c, GROUP_ORDER[2], xts)

    nc.compile()
    return nc


_SHAMT = (4 * (np.arange(P, dtype=np.int32) % 8)).reshape(P, 1)


def make_in_maps(x2d, qweight, scales, zeros, bias):
    """Per-core input maps (host-side sharding / layout prep only)."""
    in_maps = []
    for c in range(NCORES):
        sl = slice(c * NSH, (c + 1) * NSH)
        in_maps.append(
            {
                "x": x2d,
                "qw": np.ascontiguousarray(
                    np.repeat(qweight[:, sl], 8, axis=0)
                ),
                "scales": np.ascontiguousarray(scales[sl, 0]),
                "zeros": np.ascontiguousarray(zeros[sl, 0]),
                "bias": np.ascontiguousarray(bias[sl]),
                "shamt": _SHAMT,
            }
        )
    return in_maps


_NC_CACHE = {}


def _get_nc(m_rows):
    if m_rows not in _NC_CACHE:
        _NC_CACHE[m_rows] = build(m_rows)
    return _NC_CACHE[m_rows]


def run_spmd(x2d, qweight, scales, zeros, bias, trace=False, **kwargs):
    """Run on the 8 NeuronCores; returns (out2d [8192, 11008] f32, results)."""
    from concourse.bass_utils import run_bass_kernel_spmd

    m_rows = x2d.shape[0]
    nc = _get_nc(m_rows)
    in_maps = make_in_maps(x2d, qweight, scales, zeros, bias)
    res = run_bass_kernel_spmd(
        nc, in_maps, list(range(NCORES)), trace=trace, **kwargs
    )
    outs = [res.results[c]["out"] for c in range(NCORES)]
    out2d = np.concatenate(outs, axis=1)
    return out2d, res


def kernel(x, qweight, scales, zeros, bias):
    x = np.asarray(x, dtype=np.float32)
    qweight = np.asarray(qweight, dtype=np.int32)
    scales = np.asarray(scales, dtype=np.float32)
    zeros = np.asarray(zeros, dtype=np.float32)
    bias = np.asarray(bias, dtype=np.float32)

    b, s, k_in = x.shape
    x2d = np.ascontiguousarray(x.reshape(b * s, k_in))
    out2d, _ = run_spmd(x2d, qweight, scales, zeros, bias)
    return out2d.reshape(b, s, OUT)


# revision 36
# speedup vs baseline: 1.6318x; 1.1889x over previous
"""Trainium2 Bass kernel for nn_Autograd4bitQuantLinear (4-bit quant linear).

Computes out = x @ dequant4(qweight, scales, zeros) + bias where
  x:       (4, 2048, 4096) f32
  qweight: (512, 11008)    i32  (8 nibbles packed per int32 along rows)
  scales:  (11008, 1)      f32
  zeros:   (11008, 1)      f32
  bias:    (11008,)        f32
  out:     (4, 2048, 11008) f32

Strategy (tensor-parallel over 8 NeuronCores, column-sharded out_features):
  - Each core owns 1376 output columns; x is replicated.
  - On-device dequant: nibble-unpack qweight int32 (DVE shift/and with
    per-partition shift amounts), fold scale/zero in (W = q * s - z) and
    store W as bf16 [4096, 1376] resident in SBUF, split in three column
    groups (one per PSUM n-chunk). Unpack of group i is emitted right
    before the first m-chunk's chunk-i matmuls so the PE starts ~30us in
    and is never head-of-line blocked behind later unpack work on DVE.
  - x is cast f32->bf16 by a SWDGE cast-DMA into a DRAM scratch tile, then
    DMA-transposed (xbar) into SBUF as [k, m] tiles.
  - PE: out[m, n] accumulated over 32 k-tiles in PSUM (bf16 x bf16 -> f32).
  - Epilogue: psum + bias (f32, DVE) -> SBUF -> per-chunk DMA out.
  - Engine split to avoid HWDGE head-of-line blocking: sync engine issues
    only the xbar transposes; scalar engine issues qweight loads and
    output stores; gpsimd (SWDGE) does the cast + broadcast DMAs.

Host-side prep per core is layout-only: shard slicing, row-replication of
the packed qweight (np.repeat, so each SBUF partition k holds the packed
word k//8), and tiny constant vectors. All dequant arithmetic runs on
device.
"""

import sys

sys.path.insert(0, "/opt/trn_rl_repo")

import numpy as np

import concourse.bass as bass
import concourse.mybir as mybir
from concourse import bacc
from concourse.tile import TileContext
from concourse.tile_rust import add_dep_helper


dt = mybir.dt
AL = mybir.AluOpType

P = 128
IN = 4096  # contraction dim (in_features)
OUT = 11008  # out_features
M_ROWS = 8192  # 4 * 2048
NCORES = 8
NSH = OUT // NCORES  # 1376 output columns per core
KT = IN // P  # 32 k-tiles
M_CHUNK = 1024  # rows per x transpose/staging chunk
# n-chunks within the per-core shard; each must fit one PSUM bank (<=512 f32)
N_CHUNKS = ((0, 512), (512, 512), (1024, 352))
XT_BUFS = 39


def build(m_rows=M_ROWS, debug=False):
    """Build + compile the single-core Tile program (SPMD: same on all cores)."""
    assert m_rows % M_CHUNK == 0
    nc = bacc.Bacc(None, target_bir_lowering=False, debug=debug)

    x_d = nc.dram_tensor("x", [m_rows, IN], dt.float32, kind="ExternalInput")
    qw_d = nc.dram_tensor("qw", [IN, NSH], dt.int32, kind="ExternalInput")
    s_d = nc.dram_tensor("scales", [NSH], dt.float32, kind="ExternalInput")
    z_d = nc.dram_tensor("zeros", [NSH], dt.float32, kind="ExternalInput")
    b_d = nc.dram_tensor("bias", [NSH], dt.float32, kind="ExternalInput")
    shamt_d = nc.dram_tensor("shamt", [P, 1], dt.int32, kind="ExternalInput")
    out_d = nc.dram_tensor("out", [m_rows, NSH], dt.float32, kind="ExternalOutput")

    n_mchunks = m_rows // M_CHUNK
    mt_per_chunk = M_CHUNK // P

    with TileContext(nc) as tc:
        with (
            tc.tile_pool(name="singles", bufs=1) as singles,
            tc.tile_pool(name="w", bufs=KT) as wpool,
            tc.tile_pool(name="unpack", bufs=2) as upool,
            tc.tile_pool(name="xbf", bufs=2, space="DRAM") as xbfpool,
            tc.tile_pool(name="xbf0", bufs=1, space="DRAM") as xbf0pool,
            tc.tile_pool(name="xt", bufs=XT_BUFS) as xtpool,
            tc.tile_pool(name="osb", bufs=2) as opool,
            tc.tile_pool(name="ps", bufs=2, space="PSUM") as pspool,
        ):
            # ---- constants ----
            s_rep = singles.tile([P, NSH], dt.float32, tag="s_rep")
            nc.gpsimd.dma_start(out=s_rep[:], in_=s_d[None, :].to_broadcast([P, NSH]))
            z_rep = singles.tile([P, NSH], dt.float32, tag="z_rep")
            nc.gpsimd.dma_start(out=z_rep[:], in_=z_d[None, :].to_broadcast([P, NSH]))
            b_rep = singles.tile([P, NSH], dt.float32, tag="b_rep")
            nc.gpsimd.dma_start(out=b_rep[:], in_=b_d[None, :].to_broadcast([P, NSH]))
            shamt = singles.tile([P, 1], dt.int32, tag="shamt")
            nc.scalar.dma_start(out=shamt[:], in_=shamt_d[:])
            mask = singles.tile([P, 1], dt.int32, tag="mask")
            nc.vector.memset(mask[:], 15)

            # ---- W dequant: three column groups, tiles per (chunk, k) ----
            wtiles = {}  # (i, k) -> [P, w_i] bf16 tile

            def unpack_group(i):
                o, wd = N_CHUNKS[i]
                for k in range(KT):
                    qt = upool.tile([P, wd], dt.int32, tag="qt", name="qt")
                    nc.scalar.dma_start(
                        out=qt[:], in_=qw_d[k * P : (k + 1) * P, o : o + wd]
                    )
                    # nib = (qw >> shamt[p]) & 0xF (int32; bitvec can't cast)
                    nib = upool.tile([P, wd], dt.int32, tag="nib", name="nib")
                    nc.vector.scalar_tensor_tensor(
                        nib[:],
                        qt[:],
                        shamt[:, 0:1],
                        mask[:, 0:1].to_broadcast([P, wd]),
                        AL.logical_shift_right,
                        AL.bitwise_and,
                    )
                    ws = upool.tile([P, wd], dt.float32, tag="ws", name="ws")
                    nc.vector.tensor_tensor(
                        ws[:], nib[:], s_rep[:, o : o + wd], AL.mult
                    )
                    wt = wpool.tile([P, wd], dt.bfloat16, tag=f"w{i}", name=f"w{i}_{k}")
                    nc.vector.tensor_tensor(
                        wt[:], ws[:], z_rep[:, o : o + wd], AL.subtract
                    )
                    wtiles[(i, k)] = wt

            def do_mm(ps, xts, mt, k, i):
                nc.tensor.matmul(
                    ps[:],
                    xts[k][:, mt * P : (mt + 1) * P],
                    wtiles[(i, k)][:],
                    start=(k == 0),
                    stop=(k == KT - 1),
                )

            def epilogue(ps, row, i):
                o, wd = N_CHUNKS[i]
                ob = opool.tile([P, wd], dt.float32, tag=f"ob{i}", name=f"ob{i}")
                nc.vector.tensor_tensor(ob[:], ps[:], b_rep[:, o : o + wd], AL.add)
                nc.scalar.dma_start(out=out_d[row : row + P, o : o + wd], in_=ob[:])

            last_xpose = {}  # mc -> last transpose instruction of that chunk

            def load_chunk(mc):
                r0 = mc * M_CHUNK
                if mc == 0:
                    # chunk 0 casts in 8 column slices so the first transposes
                    # (and hence the PE) start ~8us in instead of ~50us
                    slices = []
                    for j in range(8):
                        t = xbf0pool.tile(
                            [M_CHUNK, 512], dt.bfloat16,
                            tag=f"xbf0_{j}", name=f"xbf0_{j}",
                        )
                        nc.gpsimd.dma_start(
                            out=t[:],
                            in_=x_d[0:M_CHUNK, j * 512 : (j + 1) * 512],
                        )
                        slices.append(t)
                    xts = []
                    for ks in range(KT):
                        xt = xtpool.tile(
                            [P, M_CHUNK], dt.bfloat16, tag="xt", name="xt"
                        )
                        j = ks // 4
                        ti = nc.sync.dma_start(
                            out=xt[:],
                            in_=slices[j][
                                :, ks * 128 - j * 512 : (ks + 1) * 128 - j * 512
                            ],
                            transpose=True,
                        )
                        xts.append(xt)
                    last_xpose[0] = ti
                    return xts
                xbf = xbfpool.tile([M_CHUNK, IN], dt.bfloat16, tag="xbf", name="xbf")
                # cast f32 -> bf16 during DMA (SWDGE), DRAM -> DRAM.
                # DRAM pool tiles are fresh allocations, so nothing throttles
                # the cast chain; without the explicit dep below all 16 casts
                # (134 MB) flood the SDMA rings at t=0 and starve chunk 0.
                ci = nc.gpsimd.dma_start(out=xbf[:], in_=x_d[r0 : r0 + M_CHUNK, :])
                if mc - 2 in last_xpose:
                    add_dep_helper(
                        ci.ins,
                        last_xpose[mc - 2].ins,
                        sync=True,
                        reason="throttle x cast chain",
                    )
                xts = []
                for ks in range(KT):
                    xt = xtpool.tile([P, M_CHUNK], dt.bfloat16, tag="xt", name="xt")
                    ti = nc.sync.dma_start(
                        out=xt[:], in_=xbf[:, ks * P : (ks + 1) * P], transpose=True
                    )
                    xts.append(xt)
                last_xpose[mc] = ti
                return xts

            # ---- first m-chunk: n-chunk-major, interleaved with unpack ----
            xts0 = load_chunk(0)
            for i in range(len(N_CHUNKS)):
                unpack_group(i)
                for mt in range(mt_per_chunk):
                    # rotate psum tags so mc0 can run 6 groups ahead of the
                    # DVE epilogues (which contend with unpack on DVE)
                    g = i * mt_per_chunk + mt
                    ps = pspool.tile(
                        [P, N_CHUNKS[i][1]], dt.float32,
                        tag=f"ps{g % 3}", name=f"ps{g % 3}",
                    )
                    for k in range(KT):
                        do_mm(ps, xts0, mt, k, i)
                    epilogue(ps, mt * P, i)

            # ---- steady state ----
            for mc in range(1, n_mchunks):
                xts = load_chunk(mc)
                for mt in range(mt_per_chunk):
                    pss = [
                        pspool.tile(
                            [P, wd], dt.float32, tag=f"ps{i}", name=f"ps{i}"
                        )
                        for i, (o, wd) in enumerate(N_CHUNKS)
                    ]
                    for k in range(KT):
                        for i in range(len(N_CHUNKS)):
                            do_mm(pss[i], xts, mt, k, i)
                    for i in range(len(N_CHUNKS)):
                        epilogue(pss[i], mc * M_CHUNK + mt * P, i)

    nc.compile()
    return nc


_SHAMT = (4 * (np.arange(P, dtype=np.int32) % 8)).reshape(P, 1)


def make_in_maps(x2d, qweight, scales, zeros, bias):
    """Per-core input maps (host-side sharding / layout prep only)."""
    in_maps = []
    for c in range(NCORES):
        sl = slice(c * NSH, (c + 1) * NSH)
        in_maps.append(
            {
                "x": x2d,
                "qw": np.ascontiguousarray(
                    np.repeat(qweight[:, sl], 8, axis=0)
                ),
                "scales": np.ascontiguousarray(scales[sl, 0]),
                "zeros": np.ascontiguousarray(zeros[sl, 0]),
                "bias": np.ascontiguousarray(bias[sl]),
                "shamt": _SHAMT,
            }
        )
    return in_maps


_NC_CACHE = {}


def _get_nc(m_rows):
    if m_rows not in _NC_CACHE:
        _NC_CACHE[m_rows] = build(m_rows)
    return _NC_CACHE[m_rows]


def run_spmd(x2d, qweight, scales, zeros, bias, trace=False, **kwargs):
    """Run on the 8 NeuronCores; returns (out2d [8192, 11008] f32, results)."""
    from concourse.bass_utils import run_bass_kernel_spmd

    m_rows = x2d.shape[0]
    nc = _get_nc(m_rows)
    in_maps = make_in_maps(x2d, qweight, scales, zeros, bias)
    res = run_bass_kernel_spmd(
        nc, in_maps, list(range(NCORES)), trace=trace, **kwargs
    )
    outs = [res.results[c]["out"] for c in range(NCORES)]
    out2d = np.concatenate(outs, axis=1)
    return out2d, res


def kernel(x, qweight, scales, zeros, bias):
    x = np.asarray(x, dtype=np.float32)
    qweight = np.asarray(qweight, dtype=np.int32)
    scales = np.asarray(scales, dtype=np.float32)
    zeros = np.asarray(zeros, dtype=np.float32)
    bias = np.asarray(bias, dtype=np.float32)

    b, s, k_in = x.shape
    x2d = np.ascontiguousarray(x.reshape(b * s, k_in))
    out2d, _ = run_spmd(x2d, qweight, scales, zeros, bias)
    return out2d.reshape(b, s, OUT)

